# revision 1
# baseline (speedup 1.0000x reference)
"""MultiHeadSelfAttention + ALiBi for Trainium2, SPMD over 8 NeuronCores.

Sharding: core c handles batch b = c // 4 and head group g = c % 4
(3 of the 12 heads, grouped so per-head ALiBi band sizes balance).
Each core computes y_partial[b] = ctx(heads_g) @ Wout[rows_g]; the host
sums the 4 partials per batch and adds bout.

Device pipeline per core (all fp32):
  1. QK^T = Wqk^T @ x^T  -> per head: Q'/8+bq into dual Q buffers, K+bk
     into K buffer.  V = x @ Wv + bv (ones column appended per head for
     softmax denominators).
  2. S^T blocks [128k x 512q]: matmul with augmented contraction rows
     carrying the attention mask bias and, off-diagonal, the exact ALiBi
     term -slope*|q-k| (linear there).  Diagonal blocks get a fused DVE
     (rel * -slope + S) pass.  exp() on ScalarE over 3-block groups,
     P^T @ V_aug accumulated in PSUM -> unnormalized ctx^T + denom row.
  3. ctx^T = ctx_u^T * (1/denom) (1/x = exp(-ln x) on ScalarE, denom
     broadcast across partitions via a K=1 matmul); y = ctx^T.T @ Wout
     rows.  Blocks where ALiBi decays attention below ~1e-7 relative are
     skipped per the BANDS table (bout is added on the host).
"""

import math
import os

import numpy as np


def _ensure_concourse():
    try:
        import concourse  # noqa: F401
    except ImportError:
        import sys

        for p in ("/opt/trn_rl_repo", "/root/.axon_site/_ro/trn_rl_repo"):
            if os.path.isdir(p) and p not in sys.path:
                sys.path.insert(0, p)


B, L, D, H, DH = 2, 2048, 768, 12, 64
KT = L // 128  # 16 k-tiles
QC = L // 512  # 4 q-chunks
NH = 3  # heads per core
N_CORES = 8
GROUP_SIZE = 3  # exp/S group size in k-tiles (3 PSUM banks)

# Per head-slot key-tile bands per q-chunk (t_lo, t_hi_exclusive).  Slot 0
# holds the wide-band heads (full attention); slots 1/2 hold heads whose
# ALiBi slope decays attention to ~exp(-25) beyond d_max = 36/slope keys,
# so blocks fully outside the band contribute < 1e-7 relative mass.
BANDS = [
    [(0, 16), (0, 16), (0, 16), (0, 16)],  # slot 0: full
    [(0, 9), (0, 13), (3, 16), (7, 16)],  # slot 1: d=576
    [(0, 6), (2, 10), (6, 14), (10, 16)],  # slot 2: d=204
]

# Head groups balanced by ALiBi band size (slopes below): each group gets
# one wide-band, one mid-band and one narrow-band head.
HEAD_GROUPS = [[4, 3, 0], [5, 2, 8], [6, 11, 9], [7, 1, 10]]

NEG_MASK = -1.0e9


def alibi_slopes(n_heads: int) -> np.ndarray:
    def slopes_pow2(n):
        start = 2 ** (-(2 ** -(math.log2(n) - 3)))
        return [start * start**i for i in range(n)]

    if math.log2(n_heads).is_integer():
        s = slopes_pow2(n_heads)
    else:
        cp = 2 ** int(math.floor(math.log2(n_heads)))
        s = slopes_pow2(cp) + slopes_pow2(2 * cp)[0::2][: n_heads - cp]
    return np.asarray(s, dtype=np.float32)


_PROGRAM_CACHE = {}
DEBUG_TAPS = False


def _build_program():
    """Build the (shared, SPMD) Bass program once."""
    if "nc" in _PROGRAM_CACHE:
        return _PROGRAM_CACHE["nc"]

    _ensure_concourse()
    import concourse.mybir as mybir
    import concourse.tile as tile
    from concourse import bacc
    from concourse.bass import ts

    f32 = mybir.dt.float32
    Exp = mybir.ActivationFunctionType.Exp
    MULT = mybir.AluOpType.mult
    ADD = mybir.AluOpType.add

    slopes = alibi_slopes(H)

    nc = bacc.Bacc(None)

    # ---- DRAM I/O ----
    xT_d = nc.dram_tensor("xT", [D, L], f32, kind="ExternalInput")
    wqk_d = nc.dram_tensor("wqk", [D, 2 * DH * NH], f32, kind="ExternalInput")
    bqk_d = nc.dram_tensor("bqk", [128, NH], f32, kind="ExternalInput")
    wv_d = nc.dram_tensor("wv", [D, DH * NH], f32, kind="ExternalInput")
    bv_d = nc.dram_tensor("bv", [1, DH * NH], f32, kind="ExternalInput")
    woutp_d = nc.dram_tensor("woutp", [256, D], f32, kind="ExternalInput")
    augqR_d = nc.dram_tensor("augqR", [3, L], f32, kind="ExternalInput")
    augqL_d = nc.dram_tensor("augqL", [3, L], f32, kind="ExternalInput")
    augk_d = nc.dram_tensor("augk", [NH, 3, L], f32, kind="ExternalInput")
    # rel4[p, j, m, q'] = -slope_j * |q' - 128m - p| (pre-scaled per slot)
    rel4_d = nc.dram_tensor("rel4", [128, NH, 4, 512], f32, kind="ExternalInput")
    y_d = nc.dram_tensor("ypart", [L, D], f32, kind="ExternalOutput")
    if DEBUG_TAPS:
        dbg_q = nc.dram_tensor("dbg_q", [67, L], f32, kind="ExternalOutput")
        dbg_k = nc.dram_tensor("dbg_k", [67, L], f32, kind="ExternalOutput")
        dbg_v = nc.dram_tensor("dbg_v", [128, NH * (DH + 1)], f32, kind="ExternalOutput")
        dbg_pt = nc.dram_tensor("dbg_pt", [128, 3 * 512], f32, kind="ExternalOutput")
        dbg_ou = nc.dram_tensor("dbg_ou", [128, 512], f32, kind="ExternalOutput")
        dbg_rec = nc.dram_tensor("dbg_rec", [64, 512], f32, kind="ExternalOutput")
        dbg_ctx = nc.dram_tensor("dbg_ctx", [128, L], f32, kind="ExternalOutput")

    with tile.TileContext(nc) as tc:
        with tc.tile_pool(name="persist", bufs=1) as pp:
            # ---- persistent SBUF ----
            wqk_sb = pp.tile([128, 6, 2 * DH * NH], f32)
            bqk_sb = pp.tile([128, NH], f32)
            wv_sb = pp.tile([128, 6, DH * NH], f32)
            bv_sb = pp.tile([1, DH * NH], f32)
            woutp_sb = pp.tile([128, 2, D], f32)
            rel4_sb = pp.tile([128, NH, 4, 512], f32)
            ones_sb = pp.tile([1, 128], f32)
            V_sb = pp.tile([128, KT, NH, DH + 1], f32)
            ctxA = pp.tile([128, L], f32)  # heads 0,1 of group
            ctxB = pp.tile([64, L], f32)  # head 2 of group
            # Per-head attention operand buffers.
            # Kbuf rows: 0-63 K^T, 64 mask, 65 s*k_idx, 66 s*1
            # QbufR rows: 0-63 Q'^T, 64 1, 65 1, 66 -q_idx
            # QbufL rows: 0-63 Q'^T, 64 1, 65 -1, 66 +q_idx
            Kbuf = [
                pp.tile([67, L], f32, tag=f"kb{j}", name=f"kb{j}") for j in range(NH)
            ]
            QbufR = [
                pp.tile([67, L], f32, tag=f"qr{j}", name=f"qr{j}") for j in range(NH)
            ]
            QbufL = [
                pp.tile([67, L], f32, tag=f"ql{j}", name=f"ql{j}") for j in range(NH)
            ]

            nc.sync.dma_start(wqk_sb[:], wqk_d.rearrange("(o p) m -> p o m", p=128))
            nc.sync.dma_start(bqk_sb[:], bqk_d[:])
            nc.sync.dma_start(wv_sb[:], wv_d.rearrange("(o p) m -> p o m", p=128))
            nc.sync.dma_start(bv_sb[:], bv_d[:])
            nc.sync.dma_start(woutp_sb[:], woutp_d.rearrange("(o p) n -> p o n", p=128))
            nc.sync.dma_start(rel4_sb[:], rel4_d[:])
            nc.vector.memset(ones_sb[:], 1.0)
            nc.gpsimd.memset(V_sb[:, :, :, DH : DH + 1], 1.0)
            for j in range(NH):
                nc.sync.dma_start(QbufR[j][64:67, :], augqR_d[:])
                nc.sync.dma_start(QbufL[j][64:67, :], augqL_d[:])
                nc.sync.dma_start(Kbuf[j][64:67, :], augk_d[j])

            # Pre-touch DMA-loaded tiles with their consuming engines so
            # later TensorScalarPtr ops carry a single sync wait (the
            # walrus TS encoding rejects multi-wait instructions).
            junk = pp.tile([1, 16], f32, name="junk")
            nc.vector.tensor_copy(junk[0:1, 0:1], bqk_sb[0:1, 0:1])
            nc.vector.tensor_copy(junk[0:1, 1:2], rel4_sb[0:1, 0, 0, 0:1])
            for j in range(NH):
                nc.vector.tensor_copy(junk[0:1, 2 + j : 3 + j], QbufR[j][64:65, 0:1])
                nc.vector.tensor_copy(junk[0:1, 5 + j : 6 + j], QbufL[j][64:65, 0:1])
                nc.vector.tensor_copy(junk[0:1, 8 + j : 9 + j], Kbuf[j][64:65, 0:1])
            junk2 = pp.tile([1, 4], f32, name="junk2")
            nc.scalar.copy(junk2[0:1, 0:1], V_sb[0:1, 0, 0, DH : DH + 1])

            # ---- stage 1: QKV projection ----
            with (
                tc.tile_pool(name="xpool", bufs=1) as xp,
                tc.tile_pool(name="ps1", bufs=4, space="PSUM") as ps1,
            ):
                xT_sb = xp.tile([128, 6, L], f32)
                for kt in range(6):
                    nc.sync.dma_start(
                        xT_sb[:, kt, :],
                        xT_d.rearrange("(o p) f -> p o f", p=128)[:, kt, :],
                    )

                # Q^T/K^T per head: PSUM [128, 512] = [Q^T_h; K^T_h] chunk
                for j in range(NH):
                    pcs = [
                        ps1.tile([128, 512], f32, tag="ps1", name=f"ps1c{c}")
                        for c in range(QC)
                    ]
                    for kt in range(6):
                        for c in range(QC):
                            nc.tensor.matmul(
                                pcs[c],
                                wqk_sb[:, kt, ts(j, 128)],
                                xT_sb[:, kt, ts(c, 512)],
                                start=(kt == 0),
                                stop=(kt == 5),
                            )
                    for c in range(QC):
                        ps = pcs[c]
                        cs = ts(c, 512)
                        nc.vector.tensor_scalar(
                            QbufR[j][0:64, cs],
                            ps[0:64, :],
                            0.125,
                            bqk_sb[0:64, j : j + 1],
                            MULT,
                            ADD,
                        )
                        nc.vector.tensor_copy(QbufL[j][0:64, cs], QbufR[j][0:64, cs])
                        nc.vector.tensor_scalar(
                            Kbuf[j][0:64, cs],
                            ps[64:128, :],
                            bqk_sb[64:128, j : j + 1],
                            None,
                            ADD,
                        )

                # V natural layout [l, d] + bias via K=1 matmul
                for lt in range(KT):
                    psv = ps1.tile([128, 512], f32, tag="ps1", name="psv")[:, : DH * NH]
                    for kt in range(6):
                        nc.tensor.matmul(
                            psv,
                            xT_sb[:, kt, ts(lt, 128)],
                            wv_sb[:, kt, :],
                            start=(kt == 0),
                            stop=False,
                        )
                    nc.tensor.matmul(
                        psv,
                        ones_sb[0:1, 0:128],
                        bv_sb[0:1, :],
                        start=False,
                        stop=True,
                    )
                    nc.scalar.copy(
                        V_sb[:, lt, :, 0:DH],
                        psv.rearrange("p (h x) -> p h x", x=DH),
                    )

            # ---- stage 2: attention ----
            with (
                tc.tile_pool(name="psS", bufs=2, space="PSUM") as psS,
                tc.tile_pool(name="psO", bufs=2, space="PSUM") as psO,
                tc.tile_pool(name="ptp", bufs=3) as ptp,
                tc.tile_pool(name="nrm", bufs=2) as nrm,
            ):
                for j in range(NH):
                    for c in range(QC):
                        cs = ts(c, 512)
                        t_lo, t_hi = BANDS[j][c]
                        out_t = psO.tile([128, 512], f32, tag="outaug")
                        for t0 in range(t_lo, t_hi, GROUP_SIZE):
                            tn = min(GROUP_SIZE, t_hi - t0)
                            st = psS.tile([128, GROUP_SIZE * 512], f32, tag="st")
                            for i in range(tn):
                                t = t0 + i
                                js = ts(i, 512)
                                m = t - 4 * c
                                if 0 <= m < 4:  # diagonal block
                                    nc.tensor.matmul(
                                        st[:, js],
                                        Kbuf[j][0:65, ts(t, 128)],
                                        QbufR[j][0:65, cs],
                                        start=True,
                                        stop=True,
                                    )
                                    nc.vector.scalar_tensor_tensor(
                                        st[:, js],
                                        rel4_sb[:, j, m, :],
                                        1.0,
                                        st[:, js],
                                        MULT,
                                        ADD,
                                    )
                                elif c > t // 4:  # strictly right of diag
                                    nc.tensor.matmul(
                                        st[:, js],
                                        Kbuf[j][0:67, ts(t, 128)],
                                        QbufR[j][0:67, cs],
                                        start=True,
                                        stop=True,
                                    )
                                else:  # strictly left
                                    nc.tensor.matmul(
                                        st[:, js],
                                        Kbuf[j][0:67, ts(t, 128)],
                                        QbufL[j][0:67, cs],
                                        start=True,
                                        stop=True,
                                    )
                            pt = ptp.tile([128, GROUP_SIZE * 512], f32, tag="pt")
                            nc.scalar.activation(
                                pt[:, : tn * 512], st[:, : tn * 512], Exp
                            )
                            if DEBUG_TAPS and j == 0 and c == 0 and g == 0:
                                nc.sync.dma_start(dbg_pt[:], pt[:])
                            for i in range(tn):
                                t = t0 + i
                                nc.tensor.matmul(
                                    out_t[0 : DH + 1, :],
                                    V_sb[:, t, j, :],
                                    pt[:, ts(i, 512)],
                                    start=(t == t_lo),
                                    stop=(t == t_hi - 1),
                                    skip_group_check=True,
                                )
                        # 1/denom = exp(-ln(denom)) on ScalarE (the custom
                        # DVE reciprocal ops misbehave under this runtime).
                        lnr = nrm.tile([1, 512], f32, tag="lnr")
                        nc.scalar.activation(
                            lnr, out_t[DH : DH + 1, :], mybir.ActivationFunctionType.Ln
                        )
                        rec = nrm.tile([1, 512], f32, tag="rec")
                        nc.scalar.activation(rec, lnr, Exp, scale=-1.0)
                        # broadcast 1/denom across 64 partitions via K=1
                        # matmul into a base-0 PSUM tile (partition-offset
                        # matmul outputs misbehave on HW), then copy out.
                        recb_ps = psS.tile(
                            [128, GROUP_SIZE * 512], f32, tag="st", name="recps"
                        )[0:64, 0:512]
                        nc.tensor.matmul(
                            recb_ps,
                            ones_sb[0:1, 0:64],
                            rec,
                            start=True,
                            stop=True,
                            skip_group_check=True,
                        )
                        recb = nrm.tile([64, 512], f32, tag="recb")
                        nc.vector.tensor_copy(recb, recb_ps)
                        if j < 2:
                            ctx_slice = ctxA[j * 64 : (j + 1) * 64, cs]
                        else:
                            ctx_slice = ctxB[0:64, cs]
                        if DEBUG_TAPS and j == 0 and c == 0:
                            ou_sb = nrm.tile([128, 512], f32, tag="ousb", name="ousb")
                            nc.vector.tensor_copy(ou_sb[0:64, :], out_t[0:64, :])
                            nc.vector.tensor_copy(ou_sb[64:128, :], out_t[64:128, :])
                            nc.sync.dma_start(dbg_ou[:], ou_sb[:])
                            nc.sync.dma_start(dbg_rec[:], recb[:])
                        nc.vector.tensor_mul(ctx_slice, out_t[0:DH, :], recb)

            if DEBUG_TAPS:
                nc.sync.dma_start(dbg_q[:], QbufR[0][:])
                nc.sync.dma_start(dbg_k[:], Kbuf[0][:])
                nc.sync.dma_start(dbg_v[:], V_sb[:, 0, :, :].rearrange("p h x -> p (h x)"))
                nc.sync.dma_start(dbg_ctx[:], ctxA[:])

            # ---- stage 3: output projection ----
            with (
                tc.tile_pool(name="ps3", bufs=2, space="PSUM") as ps3,
                tc.tile_pool(name="ysb", bufs=3) as yp,
            ):
                for lt in range(KT):
                    y = yp.tile([128, D], f32, tag="y")
                    for n0, nw in ((0, 512), (512, 256)):
                        ps = ps3.tile([128, 512], f32, tag="ps3", name="ps3t")[:, :nw]
                        nc.tensor.matmul(
                            ps,
                            ctxA[:, ts(lt, 128)],
                            woutp_sb[:, 0, n0 : n0 + nw],
                            start=True,
                            stop=False,
                        )
                        nc.tensor.matmul(
                            ps,
                            ctxB[0:64, ts(lt, 128)],
                            woutp_sb[0:64, 1, n0 : n0 + nw],
                            start=False,
                            stop=True,
                        )
                        nc.scalar.copy(y[:, n0 : n0 + nw], ps)
                    nc.sync.dma_start(y_d[ts(lt, 128), :], y)

    if not nc.is_finalized():
        nc.finalize()
    _PROGRAM_CACHE["nc"] = nc
    return nc


def _host_inputs(x, attn_mask, Wqkv, bqkv, Wout, bout):
    """Build the 8 per-core input dicts."""
    slopes = alibi_slopes(H)
    x = np.asarray(x, dtype=np.float32)
    attn_mask = np.asarray(attn_mask)
    Wqkv = np.asarray(Wqkv, dtype=np.float32)
    bqkv = np.asarray(bqkv, dtype=np.float32)
    Wout = np.asarray(Wout, dtype=np.float32)
    bout = np.asarray(bout, dtype=np.float32)

    q_idx = np.arange(L, dtype=np.float32)
    ones_row = np.ones(L, dtype=np.float32)
    augqR = np.ascontiguousarray(np.stack([ones_row, ones_row, -q_idx]))
    augqL = np.ascontiguousarray(np.stack([ones_row, -ones_row, q_idx]))

    # rel4[p, m, q'] = |q' - 128m - p|  (diagonal-block relative distance)
    p = np.arange(128, dtype=np.float32)[:, None, None]
    m = np.arange(4, dtype=np.float32)[None, :, None]
    qq = np.arange(512, dtype=np.float32)[None, None, :]
    rel4_base = np.abs(qq - 128.0 * m - p).astype(np.float32)

    in_maps = []
    for core in range(N_CORES):
        b = core // 4
        g = core % 4
        heads = HEAD_GROUPS[g]

        wqk = np.empty((D, 2 * DH * NH), np.float32)
        bqk = np.empty((128, NH), np.float32)
        wv = np.empty((D, DH * NH), np.float32)
        bv = np.empty((1, DH * NH), np.float32)
        woutp = np.zeros((256, D), np.float32)
        augk = np.empty((NH, 3, L), np.float32)
        mask_row = np.where(attn_mask[b] == 0, NEG_MASK, 0.0).astype(np.float32)
        rel4 = np.empty((128, NH, 4, 512), np.float32)
        for jj, h in enumerate(heads):
            rel4[:, jj] = -float(slopes[h]) * rel4_base
            wqk[:, jj * 128 : jj * 128 + 64] = Wqkv[:, h * DH : (h + 1) * DH]
            wqk[:, jj * 128 + 64 : (jj + 1) * 128] = Wqkv[
                :, D + h * DH : D + (h + 1) * DH
            ]
            bqk[0:64, jj] = bqkv[h * DH : (h + 1) * DH] * 0.125
            bqk[64:128, jj] = bqkv[D + h * DH : D + (h + 1) * DH]
            wv[:, jj * DH : (jj + 1) * DH] = Wqkv[:, 2 * D + h * DH : 2 * D + (h + 1) * DH]
            bv[0, jj * DH : (jj + 1) * DH] = bqkv[2 * D + h * DH : 2 * D + (h + 1) * DH]
            woutp[jj * DH : (jj + 1) * DH, :] = Wout[h * DH : (h + 1) * DH, :]
            s = float(slopes[h])
            augk[jj, 0, :] = mask_row
            augk[jj, 1, :] = s * q_idx  # s * k_idx along keys
            augk[jj, 2, :] = s
        in_maps.append(
            {
                "xT": np.ascontiguousarray(x[b].T),
                "wqk": wqk,
                "bqk": bqk,
                "wv": wv,
                "bv": bv,
                "woutp": woutp,
                "augqR": augqR,
                "augqL": augqL,
                "augk": augk,
                "rel4": rel4,
            }
        )
    return in_maps


def kernel(x, attn_mask, Wqkv, bqkv, Wout, bout):
    _ensure_concourse()
    from concourse.bass_utils import run_bass_kernel_spmd

    nc = _build_program()
    in_maps = _host_inputs(x, attn_mask, Wqkv, bqkv, Wout, bout)
    for m in in_maps:
        m.pop("ypart", None)

    res = run_bass_kernel_spmd(
        nc,
        in_maps,
        list(range(N_CORES)),
        trace=bool(os.environ.get("BASS_TRACE")),
    )
    outs = [r["ypart"] for r in res.results]
    out = np.zeros((B, L, D), np.float32)
    for core in range(N_CORES):
        out[core // 4] += outs[core]
    out += np.asarray(bout, np.float32)[None, None, :]
    if res.exec_time_ns is not None:
        kernel.last_exec_time_ns = res.exec_time_ns
    return out



# revision 2
# speedup vs baseline: 4.3935x; 4.3935x over previous
"""MultiHeadSelfAttention + ALiBi for Trainium2, SPMD over 8 NeuronCores.

Sharding: core c handles batch b = c // 4 and head group g = c % 4
(3 of the 12 heads, grouped so per-head ALiBi band sizes balance).
Each core computes y_partial[b] = ctx(heads_g) @ Wout[rows_g]; the host
sums the 4 partials per batch and adds bout.

Device pipeline per core (all fp32):
  1. QK^T = Wqk^T @ x^T  -> per head: Q'/8+bq into dual Q buffers, K+bk
     into K buffer.  V = x @ Wv + bv (ones column appended per head for
     softmax denominators).
  2. S^T blocks [128k x 512q]: matmul with augmented contraction rows
     carrying the attention mask bias and, off-diagonal, the exact ALiBi
     term -slope*|q-k| (linear there).  Diagonal blocks get a fused DVE
     (rel * -slope + S) pass.  exp() on ScalarE over 3-block groups,
     P^T @ V_aug accumulated in PSUM -> unnormalized ctx^T + denom row.
  3. ctx^T = ctx_u^T * (1/denom) (1/x = exp(-ln x) on ScalarE, denom
     broadcast across partitions via a K=1 matmul); y = ctx^T.T @ Wout
     rows.  Blocks where ALiBi decays attention below ~1e-7 relative are
     skipped per the BANDS table (bout is added on the host).
"""

import math
import os

import numpy as np


def _ensure_concourse():
    try:
        import concourse  # noqa: F401
    except ImportError:
        import sys

        for p in ("/opt/trn_rl_repo", "/root/.axon_site/_ro/trn_rl_repo"):
            if os.path.isdir(p) and p not in sys.path:
                sys.path.insert(0, p)


B, L, D, H, DH = 2, 2048, 768, 12, 64
KT = L // 128  # 16 k-tiles
QC = L // 512  # 4 q-chunks
NH = 3  # heads per core
N_CORES = 8
GROUP_SIZE = 3  # exp/S group size in k-tiles (3 PSUM banks)

# Per head-slot key-tile bands per q-chunk (t_lo, t_hi_exclusive).  Slot 0
# holds the wide-band heads (full attention); slots 1/2 hold heads whose
# ALiBi slope decays attention to ~exp(-25) beyond d_max = 36/slope keys,
# so blocks fully outside the band contribute < 1e-7 relative mass.
BANDS = [
    [(0, 16), (0, 16), (0, 16), (0, 16)],  # slot 0: full
    [(0, 9), (0, 13), (3, 16), (7, 16)],  # slot 1: d=576
    [(0, 6), (2, 10), (6, 14), (10, 16)],  # slot 2: d=204
]

# Head groups balanced by ALiBi band size (slopes below): each group gets
# one wide-band, one mid-band and one narrow-band head.
HEAD_GROUPS = [[4, 3, 0], [5, 2, 8], [6, 11, 9], [7, 1, 10]]

NEG_MASK = -1.0e9


def alibi_slopes(n_heads: int) -> np.ndarray:
    def slopes_pow2(n):
        start = 2 ** (-(2 ** -(math.log2(n) - 3)))
        return [start * start**i for i in range(n)]

    if math.log2(n_heads).is_integer():
        s = slopes_pow2(n_heads)
    else:
        cp = 2 ** int(math.floor(math.log2(n_heads)))
        s = slopes_pow2(cp) + slopes_pow2(2 * cp)[0::2][: n_heads - cp]
    return np.asarray(s, dtype=np.float32)


_PROGRAM_CACHE = {}
DEBUG_TAPS = False


def _build_program():
    """Build the (shared, SPMD) Bass program once."""
    if "nc" in _PROGRAM_CACHE:
        return _PROGRAM_CACHE["nc"]

    _ensure_concourse()
    import concourse.mybir as mybir
    import concourse.tile as tile
    from concourse import bacc
    from concourse.bass import ts

    f32 = mybir.dt.float32
    Exp = mybir.ActivationFunctionType.Exp
    MULT = mybir.AluOpType.mult
    ADD = mybir.AluOpType.add

    slopes = alibi_slopes(H)

    nc = bacc.Bacc(None)

    # ---- DRAM I/O ----
    xT_d = nc.dram_tensor("xT", [D, L], f32, kind="ExternalInput")
    wqk_d = nc.dram_tensor("wqk", [D, 2 * DH * NH], f32, kind="ExternalInput")
    bqk_d = nc.dram_tensor("bqk", [128, NH], f32, kind="ExternalInput")
    wv_d = nc.dram_tensor("wv", [D, DH * NH], f32, kind="ExternalInput")
    bv_d = nc.dram_tensor("bv", [1, DH * NH], f32, kind="ExternalInput")
    woutp_d = nc.dram_tensor("woutp", [256, D], f32, kind="ExternalInput")
    augqR_d = nc.dram_tensor("augqR", [3, L], f32, kind="ExternalInput")
    augqL_d = nc.dram_tensor("augqL", [3, L], f32, kind="ExternalInput")
    augk_d = nc.dram_tensor("augk", [NH, 3, L], f32, kind="ExternalInput")
    # rel4[p, j, m, q'] = -slope_j * |q' - 128m - p| (pre-scaled per slot)
    rel4_d = nc.dram_tensor("rel4", [128, NH, 4, 512], f32, kind="ExternalInput")
    y_d = nc.dram_tensor("ypart", [L, D], f32, kind="ExternalOutput")
    if DEBUG_TAPS:
        dbg_q = nc.dram_tensor("dbg_q", [67, L], f32, kind="ExternalOutput")
        dbg_k = nc.dram_tensor("dbg_k", [67, L], f32, kind="ExternalOutput")
        dbg_v = nc.dram_tensor("dbg_v", [128, NH * (DH + 1)], f32, kind="ExternalOutput")
        dbg_pt = nc.dram_tensor("dbg_pt", [128, 3 * 512], f32, kind="ExternalOutput")
        dbg_ou = nc.dram_tensor("dbg_ou", [128, 512], f32, kind="ExternalOutput")
        dbg_rec = nc.dram_tensor("dbg_rec", [64, 512], f32, kind="ExternalOutput")
        dbg_ctx = nc.dram_tensor("dbg_ctx", [128, L], f32, kind="ExternalOutput")

    with tile.TileContext(nc) as tc:
        with tc.tile_pool(name="persist", bufs=1) as pp:
            # ---- persistent SBUF ----
            wqk_sb = pp.tile([128, 6, 2 * DH * NH], f32)
            bqk_sb = pp.tile([128, NH], f32)
            wv_sb = pp.tile([128, 6, DH * NH], f32)
            bv_sb = pp.tile([1, DH * NH], f32)
            woutp_sb = pp.tile([128, 2, D], f32)
            rel4_sb = pp.tile([128, NH, 4, 512], f32)
            ones_sb = pp.tile([1, 128], f32)
            V_sb = pp.tile([128, KT, NH, DH + 1], f32)
            ctxA = pp.tile([128, L], f32)  # heads 0,1 of group
            ctxB = pp.tile([64, L], f32)  # head 2 of group
            # Per-head attention operand buffers.
            # Kbuf rows: 0-63 K^T, 64 mask, 65 s*k_idx, 66 s*1
            # QbufR rows: 0-63 Q'^T, 64 1, 65 1, 66 -q_idx
            # QbufL rows: 0-63 Q'^T, 64 1, 65 -1, 66 +q_idx
            Kbuf = [
                pp.tile([67, L], f32, tag=f"kb{j}", name=f"kb{j}") for j in range(NH)
            ]
            QbufR = [
                pp.tile([67, L], f32, tag=f"qr{j}", name=f"qr{j}") for j in range(NH)
            ]
            QbufL = [
                pp.tile([67, L], f32, tag=f"ql{j}", name=f"ql{j}") for j in range(NH)
            ]

            nc.sync.dma_start(wqk_sb[:], wqk_d.rearrange("(o p) m -> p o m", p=128))
            nc.sync.dma_start(bqk_sb[:], bqk_d[:])
            nc.sync.dma_start(wv_sb[:], wv_d.rearrange("(o p) m -> p o m", p=128))
            nc.sync.dma_start(bv_sb[:], bv_d[:])
            nc.sync.dma_start(woutp_sb[:], woutp_d.rearrange("(o p) n -> p o n", p=128))
            nc.sync.dma_start(rel4_sb[:], rel4_d[:])
            nc.vector.memset(ones_sb[:], 1.0)
            nc.gpsimd.memset(V_sb[:, :, :, DH : DH + 1], 1.0)
            for j in range(NH):
                nc.sync.dma_start(QbufR[j][64:67, :], augqR_d[:])
                nc.sync.dma_start(QbufL[j][64:67, :], augqL_d[:])
                nc.sync.dma_start(Kbuf[j][64:67, :], augk_d[j])

            # Pre-touch DMA-loaded tiles with their consuming engines so
            # later TensorScalarPtr ops carry a single sync wait (the
            # walrus TS encoding rejects multi-wait instructions).
            junk = pp.tile([1, 16], f32, name="junk")
            nc.vector.tensor_copy(junk[0:1, 0:1], bqk_sb[0:1, 0:1])
            nc.vector.tensor_copy(junk[0:1, 1:2], rel4_sb[0:1, 0, 0, 0:1])
            for j in range(NH):
                nc.vector.tensor_copy(junk[0:1, 2 + j : 3 + j], QbufR[j][64:65, 0:1])
                nc.vector.tensor_copy(junk[0:1, 5 + j : 6 + j], QbufL[j][64:65, 0:1])
                nc.vector.tensor_copy(junk[0:1, 8 + j : 9 + j], Kbuf[j][64:65, 0:1])
            junk2 = pp.tile([1, 4], f32, name="junk2")
            nc.scalar.copy(junk2[0:1, 0:1], V_sb[0:1, 0, 0, DH : DH + 1])

            # ---- stage 1: QKV projection ----
            with (
                tc.tile_pool(name="xpool", bufs=1) as xp,
                tc.tile_pool(name="ps1", bufs=4, space="PSUM") as ps1,
            ):
                xT_sb = xp.tile([128, 6, L], f32)
                for kt in range(6):
                    nc.sync.dma_start(
                        xT_sb[:, kt, :],
                        xT_d.rearrange("(o p) f -> p o f", p=128)[:, kt, :],
                    )

                # Q^T/K^T per head: PSUM [128, 512] = [Q^T_h; K^T_h] chunk
                for j in range(NH):
                    pcs = [
                        ps1.tile([128, 512], f32, tag="ps1", name=f"ps1c{c}")
                        for c in range(QC)
                    ]
                    for kt in range(6):
                        for c in range(QC):
                            nc.tensor.matmul(
                                pcs[c],
                                wqk_sb[:, kt, ts(j, 128)],
                                xT_sb[:, kt, ts(c, 512)],
                                start=(kt == 0),
                                stop=(kt == 5),
                            )
                    for c in range(QC):
                        ps = pcs[c]
                        cs = ts(c, 512)
                        nc.vector.tensor_scalar(
                            QbufR[j][0:64, cs],
                            ps[0:64, :],
                            0.125,
                            bqk_sb[0:64, j : j + 1],
                            MULT,
                            ADD,
                        )
                        nc.vector.tensor_copy(QbufL[j][0:64, cs], QbufR[j][0:64, cs])
                        nc.vector.tensor_scalar(
                            Kbuf[j][0:64, cs],
                            ps[64:128, :],
                            bqk_sb[64:128, j : j + 1],
                            None,
                            ADD,
                        )

                # V natural layout [l, d] + bias via K=1 matmul
                for lt in range(KT):
                    psv = ps1.tile([128, 512], f32, tag="ps1", name="psv")[:, : DH * NH]
                    for kt in range(6):
                        nc.tensor.matmul(
                            psv,
                            xT_sb[:, kt, ts(lt, 128)],
                            wv_sb[:, kt, :],
                            start=(kt == 0),
                            stop=False,
                        )
                    nc.tensor.matmul(
                        psv,
                        ones_sb[0:1, 0:128],
                        bv_sb[0:1, :],
                        start=False,
                        stop=True,
                    )
                    nc.scalar.copy(
                        V_sb[:, lt, :, 0:DH],
                        psv.rearrange("p (h x) -> p h x", x=DH),
                    )

            # ---- stage 2: attention ----
            with (
                tc.tile_pool(name="psS", bufs=2, space="PSUM") as psS,
                tc.tile_pool(name="psO", bufs=2, space="PSUM") as psO,
                tc.tile_pool(name="ptp", bufs=3) as ptp,
                tc.tile_pool(name="nrm", bufs=2) as nrm,
            ):
                for j in range(NH):
                    for c in range(QC):
                        cs = ts(c, 512)
                        t_lo, t_hi = BANDS[j][c]
                        out_t = psO.tile([128, 512], f32, tag="outaug")
                        for t0 in range(t_lo, t_hi, GROUP_SIZE):
                            tn = min(GROUP_SIZE, t_hi - t0)
                            st = psS.tile([128, GROUP_SIZE * 512], f32, tag="st")
                            for i in range(tn):
                                t = t0 + i
                                js = ts(i, 512)
                                m = t - 4 * c
                                if 0 <= m < 4:  # diagonal block
                                    nc.tensor.matmul(
                                        st[:, js],
                                        Kbuf[j][0:65, ts(t, 128)],
                                        QbufR[j][0:65, cs],
                                        start=True,
                                        stop=True,
                                    )
                                    nc.vector.scalar_tensor_tensor(
                                        st[:, js],
                                        rel4_sb[:, j, m, :],
                                        1.0,
                                        st[:, js],
                                        MULT,
                                        ADD,
                                    )
                                elif c > t // 4:  # strictly right of diag
                                    nc.tensor.matmul(
                                        st[:, js],
                                        Kbuf[j][0:67, ts(t, 128)],
                                        QbufR[j][0:67, cs],
                                        start=True,
                                        stop=True,
                                    )
                                else:  # strictly left
                                    nc.tensor.matmul(
                                        st[:, js],
                                        Kbuf[j][0:67, ts(t, 128)],
                                        QbufL[j][0:67, cs],
                                        start=True,
                                        stop=True,
                                    )
                            pt = ptp.tile([128, GROUP_SIZE * 512], f32, tag="pt")
                            nc.scalar.activation(
                                pt[:, : tn * 512], st[:, : tn * 512], Exp
                            )
                            if DEBUG_TAPS and j == 0 and c == 0 and g == 0:
                                nc.sync.dma_start(dbg_pt[:], pt[:])
                            for i in range(tn):
                                t = t0 + i
                                nc.tensor.matmul(
                                    out_t[0 : DH + 1, :],
                                    V_sb[:, t, j, :],
                                    pt[:, ts(i, 512)],
                                    start=(t == t_lo),
                                    stop=(t == t_hi - 1),
                                    skip_group_check=True,
                                )
                        # 1/denom = exp(-ln(denom)) on ScalarE (the custom
                        # DVE reciprocal ops misbehave under this runtime).
                        lnr = nrm.tile([1, 512], f32, tag="lnr")
                        nc.scalar.activation(
                            lnr, out_t[DH : DH + 1, :], mybir.ActivationFunctionType.Ln
                        )
                        rec = nrm.tile([1, 512], f32, tag="rec")
                        nc.scalar.activation(rec, lnr, Exp, scale=-1.0)
                        # broadcast 1/denom across 64 partitions via K=1
                        # matmul into a base-0 PSUM tile (partition-offset
                        # matmul outputs misbehave on HW), then copy out.
                        recb_ps = psS.tile(
                            [128, GROUP_SIZE * 512], f32, tag="st", name="recps"
                        )[0:64, 0:512]
                        nc.tensor.matmul(
                            recb_ps,
                            ones_sb[0:1, 0:64],
                            rec,
                            start=True,
                            stop=True,
                            skip_group_check=True,
                        )
                        recb = nrm.tile([64, 512], f32, tag="recb")
                        nc.vector.tensor_copy(recb, recb_ps)
                        if j < 2:
                            ctx_slice = ctxA[j * 64 : (j + 1) * 64, cs]
                        else:
                            ctx_slice = ctxB[0:64, cs]
                        if DEBUG_TAPS and j == 0 and c == 0:
                            ou_sb = nrm.tile([128, 512], f32, tag="ousb", name="ousb")
                            nc.vector.tensor_copy(ou_sb[0:64, :], out_t[0:64, :])
                            nc.vector.tensor_copy(ou_sb[64:128, :], out_t[64:128, :])
                            nc.sync.dma_start(dbg_ou[:], ou_sb[:])
                            nc.sync.dma_start(dbg_rec[:], recb[:])
                        nc.vector.tensor_mul(ctx_slice, out_t[0:DH, :], recb)

            if DEBUG_TAPS:
                nc.sync.dma_start(dbg_q[:], QbufR[0][:])
                nc.sync.dma_start(dbg_k[:], Kbuf[0][:])
                nc.sync.dma_start(dbg_v[:], V_sb[:, 0, :, :].rearrange("p h x -> p (h x)"))
                nc.sync.dma_start(dbg_ctx[:], ctxA[:])

            # ---- stage 3: output projection ----
            with (
                tc.tile_pool(name="ps3", bufs=2, space="PSUM") as ps3,
                tc.tile_pool(name="ysb", bufs=3) as yp,
            ):
                for lt in range(KT):
                    y = yp.tile([128, D], f32, tag="y")
                    for n0, nw in ((0, 512), (512, 256)):
                        ps = ps3.tile([128, 512], f32, tag="ps3", name="ps3t")[:, :nw]
                        nc.tensor.matmul(
                            ps,
                            ctxA[:, ts(lt, 128)],
                            woutp_sb[:, 0, n0 : n0 + nw],
                            start=True,
                            stop=False,
                        )
                        nc.tensor.matmul(
                            ps,
                            ctxB[0:64, ts(lt, 128)],
                            woutp_sb[0:64, 1, n0 : n0 + nw],
                            start=False,
                            stop=True,
                        )
                        nc.scalar.copy(y[:, n0 : n0 + nw], ps)
                    nc.sync.dma_start(y_d[ts(lt, 128), :], y)

    if not nc.is_finalized():
        nc.finalize()
    _PROGRAM_CACHE["nc"] = nc
    return nc


def _host_inputs(x, attn_mask, Wqkv, bqkv, Wout, bout):
    """Build the 8 per-core input dicts."""
    slopes = alibi_slopes(H)
    x = np.asarray(x, dtype=np.float32)
    attn_mask = np.asarray(attn_mask)
    Wqkv = np.asarray(Wqkv, dtype=np.float32)
    bqkv = np.asarray(bqkv, dtype=np.float32)
    Wout = np.asarray(Wout, dtype=np.float32)
    bout = np.asarray(bout, dtype=np.float32)

    q_idx = np.arange(L, dtype=np.float32)
    ones_row = np.ones(L, dtype=np.float32)
    augqR = np.ascontiguousarray(np.stack([ones_row, ones_row, -q_idx]))
    augqL = np.ascontiguousarray(np.stack([ones_row, -ones_row, q_idx]))

    # rel4[p, m, q'] = |q' - 128m - p|  (diagonal-block relative distance)
    p = np.arange(128, dtype=np.float32)[:, None, None]
    m = np.arange(4, dtype=np.float32)[None, :, None]
    qq = np.arange(512, dtype=np.float32)[None, None, :]
    rel4_base = np.abs(qq - 128.0 * m - p).astype(np.float32)

    in_maps = []
    for core in range(N_CORES):
        b = core // 4
        g = core % 4
        heads = HEAD_GROUPS[g]

        wqk = np.empty((D, 2 * DH * NH), np.float32)
        bqk = np.empty((128, NH), np.float32)
        wv = np.empty((D, DH * NH), np.float32)
        bv = np.empty((1, DH * NH), np.float32)
        woutp = np.zeros((256, D), np.float32)
        augk = np.empty((NH, 3, L), np.float32)
        mask_row = np.where(attn_mask[b] == 0, NEG_MASK, 0.0).astype(np.float32)
        rel4 = np.empty((128, NH, 4, 512), np.float32)
        for jj, h in enumerate(heads):
            rel4[:, jj] = -float(slopes[h]) * rel4_base
            wqk[:, jj * 128 : jj * 128 + 64] = Wqkv[:, h * DH : (h + 1) * DH]
            wqk[:, jj * 128 + 64 : (jj + 1) * 128] = Wqkv[
                :, D + h * DH : D + (h + 1) * DH
            ]
            bqk[0:64, jj] = bqkv[h * DH : (h + 1) * DH] * 0.125
            bqk[64:128, jj] = bqkv[D + h * DH : D + (h + 1) * DH]
            wv[:, jj * DH : (jj + 1) * DH] = Wqkv[:, 2 * D + h * DH : 2 * D + (h + 1) * DH]
            bv[0, jj * DH : (jj + 1) * DH] = bqkv[2 * D + h * DH : 2 * D + (h + 1) * DH]
            woutp[jj * DH : (jj + 1) * DH, :] = Wout[h * DH : (h + 1) * DH, :]
            s = float(slopes[h])
            augk[jj, 0, :] = mask_row
            augk[jj, 1, :] = s * q_idx  # s * k_idx along keys
            augk[jj, 2, :] = s
        in_maps.append(
            {
                "xT": np.ascontiguousarray(x[b].T),
                "wqk": wqk,
                "bqk": bqk,
                "wv": wv,
                "bv": bv,
                "woutp": woutp,
                "augqR": augqR,
                "augqL": augqL,
                "augk": augk,
                "rel4": rel4,
            }
        )
    return in_maps


def kernel(x, attn_mask, Wqkv, bqkv, Wout, bout):
    _ensure_concourse()
    from concourse.bass_utils import run_bass_kernel_spmd

    nc = _build_program()
    in_maps = _host_inputs(x, attn_mask, Wqkv, bqkv, Wout, bout)
    for m in in_maps:
        m.pop("ypart", None)

    res = run_bass_kernel_spmd(
        nc,
        in_maps,
        list(range(N_CORES)),
        trace=bool(os.environ.get("BASS_TRACE")),
    )
    outs = [r["ypart"] for r in res.results]
    out = np.zeros((B, L, D), np.float32)
    for core in range(N_CORES):
        out[core // 4] += outs[core]
    out += np.asarray(bout, np.float32)[None, None, :]
    if res.exec_time_ns is not None:
        kernel.last_exec_time_ns = res.exec_time_ns
    kernel.last_result = res
    return out



# revision 11
# speedup vs baseline: 10.5065x; 2.3914x over previous
"""MultiHeadSelfAttention + ALiBi for Trainium2, SPMD over 8 NeuronCores.

Sharding: core c handles batch b = c // 4 and head group g = c % 4
(3 of the 12 heads, one per ALiBi band class so per-core work balances).
Each core computes y_partial[b] = ctx(heads_g) @ Wout[rows_g]; the host
sums the 4 partials per batch and adds bout.

Device pipeline per core:
  1. QKV projection in bf16 (weights/x pre-cast on host).  Q'/8+bq and
     K+bk written as float32r into per-head attention operand buffers;
     V (+ones column for softmax denominators) kept in bf16.
  2. S^T blocks [128k x 512q] as float32r matmuls with 3 augmented
     contraction rows carrying the attention-mask bias and, off the
     diagonal, the exact ALiBi term -slope*|q-k| (linear there, indices
     re-centered at 1024 to bound fp32r rounding).  Diagonal blocks get
     a fused DVE (rel * -slope + S) pass instead.  exp() on ScalarE over
     3-block groups -> bf16 P^T; P^T @ V_aug (bf16) accumulated in PSUM
     -> unnormalized ctx^T + denominator row.  Per-slot key-tile bands
     skip blocks where ALiBi decays attention below ~e^-18.
  3. Unnormalized ctx^T (bf16) + denominators staged to SBUF; one
     batched Ln + Exp(-x) pass produces 1/denom (single ACT table set);
     reciprocals broadcast across partitions via K=1 matmuls; ctx
     normalized in place; y = ctx^T.T @ Wout rows (bf16) -> fp32 out.
"""

import math
import os

import numpy as np


def _ensure_concourse():
    try:
        import concourse  # noqa: F401
    except ImportError:
        import sys

        for p in ("/opt/trn_rl_repo", "/root/.axon_site/_ro/trn_rl_repo"):
            if os.path.isdir(p) and p not in sys.path:
                sys.path.insert(0, p)


B, L, D, H, DH = 2, 2048, 768, 12, 64
KT = L // 128  # 16 k-tiles
QC = L // 512  # 4 q-chunks
NH = 3  # heads per core
N_CORES = 8
GROUP_SIZE = 3  # exp/S group size in k-tiles (3 PSUM banks)

NEG_MASK = -1.0e9
QCENTER = 1024.0  # index re-centering for fp32r aug rows

# Per head-slot key-tile bands per q-chunk (t_lo, t_hi_exclusive).  Heads
# are assigned to slots by band class; band d satisfies e^(-slope*d) <=
# e^-18 for every head in the slot, so skipped blocks contribute < 3e-5
# relative mass.  Slot 0: heads {7,6,5,4} (widest -> full); slot 1:
# heads {3,2,11,1} (d=288); slot 2: heads {10,9,0,8} (d=102).
BANDS = [
    [(0, 16), (0, 16), (0, 16), (0, 16)],  # slot 0: full
    [(0, 7), (1, 11), (5, 15), (9, 16)],  # slot 1: d=288
    [(0, 5), (3, 9), (7, 13), (11, 16)],  # slot 2: d=102
]

# One head per band class per group -> identical program on all cores.
HEAD_GROUPS = [[7, 3, 10], [6, 2, 9], [5, 11, 0], [4, 1, 8]]


def alibi_slopes(n_heads: int) -> np.ndarray:
    def slopes_pow2(n):
        start = 2 ** (-(2 ** -(math.log2(n) - 3)))
        return [start * start**i for i in range(n)]

    if math.log2(n_heads).is_integer():
        s = slopes_pow2(n_heads)
    else:
        cp = 2 ** int(math.floor(math.log2(n_heads)))
        s = slopes_pow2(cp) + slopes_pow2(2 * cp)[0::2][: n_heads - cp]
    return np.asarray(s, dtype=np.float32)


_PROGRAM_CACHE = {}


def _build_program():
    """Build the (shared, SPMD) Bass program once."""
    if "nc" in _PROGRAM_CACHE:
        return _PROGRAM_CACHE["nc"]

    _ensure_concourse()
    import concourse.mybir as mybir
    import concourse.tile as tile
    from concourse import bacc
    from concourse.bass import ts

    f32 = mybir.dt.float32
    f32r = mybir.dt.float32r
    bf16 = mybir.dt.bfloat16
    Exp = mybir.ActivationFunctionType.Exp
    Ln = mybir.ActivationFunctionType.Ln
    MULT = mybir.AluOpType.mult
    ADD = mybir.AluOpType.add

    nc = bacc.Bacc(None)

    # ---- DRAM I/O ----
    xT_d = nc.dram_tensor("xT", [D, L], bf16, kind="ExternalInput")
    wqk_d = nc.dram_tensor("wqk", [D, 2 * DH * NH], bf16, kind="ExternalInput")
    bqk_d = nc.dram_tensor("bqk", [128, NH], f32, kind="ExternalInput")
    wv_d = nc.dram_tensor("wv", [D, DH * NH], bf16, kind="ExternalInput")
    bv_d = nc.dram_tensor("bv", [1, DH * NH], bf16, kind="ExternalInput")
    woutp_d = nc.dram_tensor("woutp", [256, D], bf16, kind="ExternalInput")
    augqR_d = nc.dram_tensor("augqR", [3, L], f32r, kind="ExternalInput")
    augqL_d = nc.dram_tensor("augqL", [3, L], f32r, kind="ExternalInput")
    augk_d = nc.dram_tensor("augk", [NH, 3, L], f32r, kind="ExternalInput")
    # rel4[p, j, m, q'] = -slope_j * |q' - 128m - p| for diagonal blocks
    rel4_d = nc.dram_tensor("rel4", [128, NH, 4, 512], f32, kind="ExternalInput")
    y_d = nc.dram_tensor("ypart", [L, D], f32, kind="ExternalOutput")

    with tile.TileContext(nc) as tc:
        with tc.tile_pool(name="persist", bufs=1) as pp:
            # ---- persistent SBUF ----
            wqk_sb = pp.tile([128, 6, 2 * DH * NH], bf16)
            bqk_sb = pp.tile([128, NH], f32)
            wv_sb = pp.tile([128, 6, DH * NH], bf16)
            bv_sb = pp.tile([1, DH * NH], bf16)
            woutp_sb = pp.tile([128, 2, D], bf16)
            rel4_sb = pp.tile([128, NH, 4, 512], f32)
            ones_sb = pp.tile([1, 128], bf16)
            V_sb = pp.tile([128, KT, NH, DH + 1], bf16)
            ctxA = pp.tile([128, L], bf16)  # heads 0,1 of group (unnorm ctx^T)
            ctxB = pp.tile([64, L], bf16)  # head 2 of group
            den_sb = pp.tile([1, NH, L], f32)  # softmax denominators
            rec_sb = pp.tile([1, NH, L], bf16)  # 1/denominators
            # Per-head attention operand buffers (fp32r).
            # Kbuf rows: 0-63 K^T, 64 mask, 65 s*(k-1024), 66 s
            # QbufR rows: 0-63 Q'^T, 64 1, 65 1, 66 -(q-1024)
            # QbufL rows: 0-63 Q'^T, 64 1, 65 -1, 66 +(q-1024)
            Kbuf = [
                pp.tile([67, L], f32r, tag=f"kb{j}", name=f"kb{j}") for j in range(NH)
            ]
            QbufR = [
                pp.tile([67, L], f32r, tag=f"qr{j}", name=f"qr{j}") for j in range(NH)
            ]
            QbufL = [
                pp.tile([67, L], f32r, tag=f"ql{j}", name=f"ql{j}") for j in range(NH)
            ]

            nc.sync.dma_start(wqk_sb[:], wqk_d.rearrange("(o p) m -> p o m", p=128))
            nc.sync.dma_start(bqk_sb[:], bqk_d[:])
            nc.sync.dma_start(wv_sb[:], wv_d.rearrange("(o p) m -> p o m", p=128))
            nc.sync.dma_start(bv_sb[:], bv_d[:])
            nc.sync.dma_start(woutp_sb[:], woutp_d.rearrange("(o p) n -> p o n", p=128))
            nc.sync.dma_start(rel4_sb[:], rel4_d[:])
            nc.vector.memset(ones_sb[:], 1.0)
            nc.gpsimd.memset(V_sb[:, :, :, DH : DH + 1], 1.0)
            for j in range(NH):
                nc.sync.dma_start(QbufR[j][64:67, :], augqR_d[:])
                nc.sync.dma_start(QbufL[j][64:67, :], augqL_d[:])
                nc.sync.dma_start(Kbuf[j][64:67, :], augk_d[j])

            # Pre-touch DMA/memset-filled tiles with the engine that later
            # writes other rows of the same tile, so those writes carry a
            # single sync wait (walrus TS encoding rejects multi-wait).
            junk = pp.tile([1, 16], f32, name="junk")
            nc.vector.tensor_copy(junk[0:1, 0:1], bqk_sb[0:1, 0:1])
            nc.vector.tensor_copy(junk[0:1, 1:2], rel4_sb[0:1, 0, 0, 0:1])
            junkr = pp.tile([1, 16], f32r, name="junkr")
            for j in range(NH):
                nc.vector.tensor_copy(junkr[0:1, 2 + j : 3 + j], QbufR[j][64:65, 0:1])
                nc.vector.tensor_copy(junkr[0:1, 5 + j : 6 + j], QbufL[j][64:65, 0:1])
                nc.vector.tensor_copy(junkr[0:1, 8 + j : 9 + j], Kbuf[j][64:65, 0:1])
            junkb = pp.tile([1, 4], bf16, name="junkb")
            nc.vector.tensor_copy(junkb[0:1, 0:1], V_sb[0:1, 0, 0, DH : DH + 1])

            # ---- stage 1: QKV projection (bf16 matmuls) ----
            with (
                tc.tile_pool(name="xpool", bufs=1) as xp,
                tc.tile_pool(name="ps1", bufs=4, space="PSUM") as ps1,
            ):
                xT_sb = xp.tile([128, 6, L], bf16)
                for kt in range(6):
                    nc.sync.dma_start(
                        xT_sb[:, kt, :],
                        xT_d.rearrange("(o p) f -> p o f", p=128)[:, kt, :],
                    )

                # Q^T/K^T per head: PSUM [128, 512] = [Q^T_h/8; K^T_h] chunk
                # (the 1/8 scale is folded into wqk/bqk host-side)
                for j in range(NH):
                    pcs = [
                        ps1.tile([128, 512], f32, tag="ps1", name=f"ps1c{c}")
                        for c in range(QC)
                    ]
                    for kt in range(6):
                        for c in range(QC):
                            nc.tensor.matmul(
                                pcs[c],
                                wqk_sb[:, kt, ts(j, 128)],
                                xT_sb[:, kt, ts(c, 512)],
                                start=(kt == 0),
                                stop=(kt == 5),
                            )
                    for c in range(QC):
                        ps = pcs[c]
                        cs = ts(c, 512)
                        nc.vector.tensor_scalar(
                            QbufR[j][0:64, cs],
                            ps[0:64, :],
                            bqk_sb[0:64, j : j + 1],
                            None,
                            ADD,
                        )
                        nc.vector.tensor_copy(QbufL[j][0:64, cs], QbufR[j][0:64, cs])
                        nc.vector.tensor_scalar(
                            Kbuf[j][0:64, cs],
                            ps[64:128, :],
                            bqk_sb[64:128, j : j + 1],
                            None,
                            ADD,
                        )

                # V natural layout [l, d] + bias via K=1 matmul
                for lt in range(KT):
                    psv = ps1.tile([128, 512], f32, tag="ps1", name="psv")[:, : DH * NH]
                    for kt in range(6):
                        nc.tensor.matmul(
                            psv,
                            xT_sb[:, kt, ts(lt, 128)],
                            wv_sb[:, kt, :],
                            start=(kt == 0),
                            stop=False,
                        )
                    nc.tensor.matmul(
                        psv,
                        ones_sb[0:1, 0:128],
                        bv_sb[0:1, :],
                        start=False,
                        stop=True,
                    )
                    nc.vector.tensor_copy(
                        V_sb[:, lt, :, 0:DH],
                        psv.rearrange("p (h x) -> p h x", x=DH),
                    )

            # ---- stage 2: attention ----
            with (
                tc.tile_pool(name="psS", bufs=2, space="PSUM") as psS,
                tc.tile_pool(name="psO", bufs=2, space="PSUM") as psO,
                tc.tile_pool(name="ptp", bufs=3) as ptp,
            ):
                for j in range(NH):
                    for c in range(QC):
                        cs = ts(c, 512)
                        t_lo, t_hi = BANDS[j][c]
                        out_t = psO.tile([128, 512], f32, tag="outaug")
                        for t0 in range(t_lo, t_hi, GROUP_SIZE):
                            tn = min(GROUP_SIZE, t_hi - t0)
                            st = psS.tile([128, GROUP_SIZE * 512], f32, tag="st")
                            for i in range(tn):
                                t = t0 + i
                                js = ts(i, 512)
                                m = t - 4 * c
                                if 0 <= m < 4:  # diagonal block
                                    nc.tensor.matmul(
                                        st[:, js],
                                        Kbuf[j][0:65, ts(t, 128)],
                                        QbufR[j][0:65, cs],
                                        start=True,
                                        stop=True,
                                    )
                                    nc.vector.scalar_tensor_tensor(
                                        st[:, js],
                                        rel4_sb[:, j, m, :],
                                        1.0,
                                        st[:, js],
                                        MULT,
                                        ADD,
                                    )
                                elif c > t // 4:  # k < q: aug gives -s*(q-k)
                                    nc.tensor.matmul(
                                        st[:, js],
                                        Kbuf[j][0:67, ts(t, 128)],
                                        QbufR[j][0:67, cs],
                                        start=True,
                                        stop=True,
                                    )
                                else:  # k > q: aug gives -s*(k-q)
                                    nc.tensor.matmul(
                                        st[:, js],
                                        Kbuf[j][0:67, ts(t, 128)],
                                        QbufL[j][0:67, cs],
                                        start=True,
                                        stop=True,
                                    )
                            pt = ptp.tile([128, GROUP_SIZE * 512], bf16, tag="pt")
                            nc.scalar.activation(
                                pt[:, : tn * 512], st[:, : tn * 512], Exp
                            )
                            for i in range(tn):
                                t = t0 + i
                                nc.tensor.matmul(
                                    out_t[0 : DH + 1, :],
                                    V_sb[:, t, j, :],
                                    pt[:, ts(i, 512)],
                                    start=(t == t_lo),
                                    stop=(t == t_hi - 1),
                                    skip_group_check=True,
                                )
                        # stage unnormalized ctx + denominator to SBUF
                        if j < 2:
                            ctx_slice = ctxA[j * 64 : (j + 1) * 64, cs]
                        else:
                            ctx_slice = ctxB[0:64, cs]
                        nc.vector.tensor_copy(ctx_slice, out_t[0:DH, :])
                        nc.vector.tensor_copy(
                            den_sb[0:1, j, cs], out_t[DH : DH + 1, :]
                        )

            # ---- stage 3: normalization + output projection ----
            with (
                tc.tile_pool(name="ps3", bufs=2, space="PSUM") as ps3,
                tc.tile_pool(name="psR", bufs=2, space="PSUM") as psR,
                tc.tile_pool(name="nrm", bufs=2) as nrm,
                tc.tile_pool(name="ysb", bufs=3) as yp,
            ):
                # 1/denom = exp(-ln(denom)), one batched pass (in-place Ln;
                # Ln and Exp share the natural_log_exp ACT table set).
                nc.scalar.activation(den_sb[:], den_sb[:], Ln)
                nc.scalar.activation(rec_sb[:], den_sb[:], Exp, scale=-1.0)

                # broadcast 1/denom across 64 partitions via K=1 matmuls,
                # then normalize ctx in place.
                for j in range(NH):
                    for c in range(QC):
                        cs = ts(c, 512)
                        recb_ps = psR.tile([64, 512], f32, tag="recb")
                        nc.tensor.matmul(
                            recb_ps,
                            ones_sb[0:1, 0:64],
                            rec_sb[0:1, j, cs],
                            start=True,
                            stop=True,
                        )
                        if j < 2:
                            ctx_slice = ctxA[j * 64 : (j + 1) * 64, cs]
                        else:
                            ctx_slice = ctxB[0:64, cs]
                        nc.vector.tensor_mul(ctx_slice, ctx_slice, recb_ps)

                for lt in range(KT):
                    y = yp.tile([128, D], f32, tag="y")
                    for n0, nw in ((0, 512), (512, 256)):
                        ps = ps3.tile([128, 512], f32, tag="ps3", name="ps3t")[:, :nw]
                        nc.tensor.matmul(
                            ps,
                            ctxA[:, ts(lt, 128)],
                            woutp_sb[:, 0, n0 : n0 + nw],
                            start=True,
                            stop=False,
                        )
                        nc.tensor.matmul(
                            ps,
                            ctxB[0:64, ts(lt, 128)],
                            woutp_sb[0:64, 1, n0 : n0 + nw],
                            start=False,
                            stop=True,
                        )
                        if n0 == 0:
                            nc.scalar.copy(y[:, n0 : n0 + nw], ps)
                        else:
                            nc.vector.tensor_copy(y[:, n0 : n0 + nw], ps)
                    nc.sync.dma_start(y_d[ts(lt, 128), :], y)

    if not nc.is_finalized():
        nc.finalize()
    _PROGRAM_CACHE["nc"] = nc
    return nc


def _host_inputs(x, attn_mask, Wqkv, bqkv, Wout, bout):
    """Build the 8 per-core input dicts."""
    import ml_dtypes

    bf16 = ml_dtypes.bfloat16
    slopes = alibi_slopes(H)
    x = np.asarray(x, dtype=np.float32)
    attn_mask = np.asarray(attn_mask)
    Wqkv = np.asarray(Wqkv, dtype=np.float32)
    bqkv = np.asarray(bqkv, dtype=np.float32)
    Wout = np.asarray(Wout, dtype=np.float32)
    bout = np.asarray(bout, dtype=np.float32)

    q_idx = np.arange(L, dtype=np.float32)
    ones_row = np.ones(L, dtype=np.float32)
    qrel = q_idx - QCENTER
    augqR = np.ascontiguousarray(np.stack([ones_row, ones_row, -qrel]))
    augqL = np.ascontiguousarray(np.stack([ones_row, -ones_row, qrel]))

    # rel4[p, m, q'] = |q' - 128m - p|  (diagonal-block relative distance)
    p = np.arange(128, dtype=np.float32)[:, None, None]
    m = np.arange(4, dtype=np.float32)[None, :, None]
    qq = np.arange(512, dtype=np.float32)[None, None, :]
    rel4_base = np.abs(qq - 128.0 * m - p).astype(np.float32)

    in_maps = []
    for core in range(N_CORES):
        b = core // 4
        g = core % 4
        heads = HEAD_GROUPS[g]

        wqk = np.empty((D, 2 * DH * NH), np.float32)
        bqk = np.empty((128, NH), np.float32)
        wv = np.empty((D, DH * NH), np.float32)
        bv = np.empty((1, DH * NH), np.float32)
        woutp = np.zeros((256, D), np.float32)
        augk = np.empty((NH, 3, L), np.float32)
        mask_row = np.where(attn_mask[b] == 0, NEG_MASK, 0.0).astype(np.float32)
        rel4 = np.empty((128, NH, 4, 512), np.float32)
        for jj, h in enumerate(heads):
            rel4[:, jj] = -float(slopes[h]) * rel4_base
            # Q half pre-scaled by 1/8 = 1/sqrt(DH)
            wqk[:, jj * 128 : jj * 128 + 64] = Wqkv[:, h * DH : (h + 1) * DH] * 0.125
            wqk[:, jj * 128 + 64 : (jj + 1) * 128] = Wqkv[
                :, D + h * DH : D + (h + 1) * DH
            ]
            bqk[0:64, jj] = bqkv[h * DH : (h + 1) * DH] * 0.125
            bqk[64:128, jj] = bqkv[D + h * DH : D + (h + 1) * DH]
            wv[:, jj * DH : (jj + 1) * DH] = Wqkv[
                :, 2 * D + h * DH : 2 * D + (h + 1) * DH
            ]
            bv[0, jj * DH : (jj + 1) * DH] = bqkv[2 * D + h * DH : 2 * D + (h + 1) * DH]
            woutp[jj * DH : (jj + 1) * DH, :] = Wout[h * DH : (h + 1) * DH, :]
            s = float(slopes[h])
            augk[jj, 0, :] = mask_row
            augk[jj, 1, :] = s * qrel  # s * (k_idx - 1024)
            augk[jj, 2, :] = s
        in_maps.append(
            {
                "xT": np.ascontiguousarray(x[b].T).astype(bf16),
                "wqk": wqk.astype(bf16),
                "bqk": bqk,
                "wv": wv.astype(bf16),
                "bv": bv.astype(bf16),
                "woutp": woutp.astype(bf16),
                "augqR": augqR,
                "augqL": augqL,
                "augk": augk,
                "rel4": rel4,
            }
        )
    return in_maps


def kernel(x, attn_mask, Wqkv, bqkv, Wout, bout):
    _ensure_concourse()
    from concourse.bass_utils import run_bass_kernel_spmd

    nc = _build_program()
    in_maps = _host_inputs(x, attn_mask, Wqkv, bqkv, Wout, bout)

    res = run_bass_kernel_spmd(
        nc,
        in_maps,
        list(range(N_CORES)),
        trace=bool(os.environ.get("BASS_TRACE")),
    )
    outs = [r["ypart"] for r in res.results]
    out = np.zeros((B, L, D), np.float32)
    for core in range(N_CORES):
        out[core // 4] += outs[core]
    out += np.asarray(bout, np.float32)[None, None, :]
    if res.exec_time_ns is not None:
        kernel.last_exec_time_ns = res.exec_time_ns
    kernel.last_result = res
    return out


# revision 23
# speedup vs baseline: 10.9373x; 1.0410x over previous
"""MultiHeadSelfAttention + ALiBi for Trainium2, SPMD over 8 NeuronCores.

Sharding: core c handles batch b = c // 4 and head group g = c % 4
(3 of the 12 heads, one per ALiBi band class so per-core work balances).
Each core computes y_partial[b] = ctx(heads_g) @ Wout[rows_g]; the host
sums the 4 partials per batch and adds bout.

Device pipeline per core:
  1. QKV projection in bf16 (weights/x pre-cast on host).  Q'/8+bq and
     K+bk written as float32r into per-head attention operand buffers;
     V (+ones column for softmax denominators) kept in bf16.
  2. S^T blocks [128k x 512q] as float32r matmuls with 3 augmented
     contraction rows carrying the attention-mask bias and, off the
     diagonal, the exact ALiBi term -slope*|q-k| (linear there, indices
     re-centered at 1024 to bound fp32r rounding).  Diagonal blocks get
     a fused DVE (rel * -slope + S) pass instead.  exp() on ScalarE over
     3-block groups -> bf16 P^T; P^T @ V_aug (bf16) accumulated in PSUM
     -> unnormalized ctx^T + denominator row.  Per-slot key-tile bands
     skip blocks where ALiBi decays attention below ~e^-18.
  3. Unnormalized ctx^T (bf16) + denominators staged to SBUF; one
     batched Ln + Exp(-x) pass produces 1/denom (single ACT table set);
     reciprocals broadcast across partitions via K=1 matmuls; ctx
     normalized in place; y = ctx^T.T @ Wout rows (bf16) -> fp32 out.
"""

import math
import os

import numpy as np


def _ensure_concourse():
    try:
        import concourse  # noqa: F401
    except ImportError:
        import sys

        for p in ("/opt/trn_rl_repo", "/root/.axon_site/_ro/trn_rl_repo"):
            if os.path.isdir(p) and p not in sys.path:
                sys.path.insert(0, p)


B, L, D, H, DH = 2, 2048, 768, 12, 64
KT = L // 128  # 16 k-tiles
QC = L // 512  # 4 q-chunks
NH = 3  # heads per core
N_CORES = 8
GROUP_SIZE = 3  # exp/S group size in k-tiles (3 PSUM banks)

NEG_MASK = -1.0e9
QCENTER = 1024.0  # index re-centering for fp32r aug rows

# Per head-slot key-tile bands per q-chunk (t_lo, t_hi_exclusive).  Heads
# are assigned to slots by band class; band d satisfies e^(-slope*d) <=
# e^-18 for every head in the slot, so skipped blocks contribute < 3e-5
# relative mass.  Slot 0: heads {7,6,5,4} (widest -> full); slot 1:
# heads {3,2,11,1} (d=288); slot 2: heads {10,9,0,8} (d=102).
BANDS = [
    [(0, 16), (0, 16), (0, 16), (0, 16)],  # slot 0: full
    [(0, 7), (1, 11), (5, 15), (9, 16)],  # slot 1: d=288
    [(0, 5), (3, 9), (7, 13), (11, 16)],  # slot 2: d=102
]

# One head per band class per group -> identical program on all cores.
HEAD_GROUPS = [[7, 3, 10], [6, 2, 9], [5, 11, 0], [4, 1, 8]]


def alibi_slopes(n_heads: int) -> np.ndarray:
    def slopes_pow2(n):
        start = 2 ** (-(2 ** -(math.log2(n) - 3)))
        return [start * start**i for i in range(n)]

    if math.log2(n_heads).is_integer():
        s = slopes_pow2(n_heads)
    else:
        cp = 2 ** int(math.floor(math.log2(n_heads)))
        s = slopes_pow2(cp) + slopes_pow2(2 * cp)[0::2][: n_heads - cp]
    return np.asarray(s, dtype=np.float32)


_PROGRAM_CACHE = {}


def _build_program():
    """Build the (shared, SPMD) Bass program once."""
    if "nc" in _PROGRAM_CACHE:
        return _PROGRAM_CACHE["nc"]

    _ensure_concourse()
    import concourse.mybir as mybir
    import concourse.tile as tile
    from concourse import bacc
    from concourse.bass import ts

    f32 = mybir.dt.float32
    f32r = mybir.dt.float32r
    bf16 = mybir.dt.bfloat16
    Exp = mybir.ActivationFunctionType.Exp
    Ln = mybir.ActivationFunctionType.Ln
    MULT = mybir.AluOpType.mult
    ADD = mybir.AluOpType.add

    nc = bacc.Bacc(None)

    # ---- DRAM I/O ----
    xT_d = nc.dram_tensor("xT", [D, L], bf16, kind="ExternalInput")
    wqk_d = nc.dram_tensor("wqk", [D, 2 * DH * NH], bf16, kind="ExternalInput")
    bqk_d = nc.dram_tensor("bqk", [128, NH], f32, kind="ExternalInput")
    wv_d = nc.dram_tensor("wv", [D, DH * NH], bf16, kind="ExternalInput")
    bv_d = nc.dram_tensor("bv", [1, DH * NH], bf16, kind="ExternalInput")
    woutp_d = nc.dram_tensor("woutp", [256, D], bf16, kind="ExternalInput")
    augqR_d = nc.dram_tensor("augqR", [3, L], f32r, kind="ExternalInput")
    augqL_d = nc.dram_tensor("augqL", [3, L], f32r, kind="ExternalInput")
    augk_d = nc.dram_tensor("augk", [NH, 3, L], f32r, kind="ExternalInput")
    # rel4[p, j, m, q'] = -slope_j * |q' - 128m - p| for diagonal blocks
    rel4_d = nc.dram_tensor("rel4", [128, NH, 4, 512], f32, kind="ExternalInput")
    y_d = nc.dram_tensor("ypart", [L, D], f32, kind="ExternalOutput")

    with tile.TileContext(nc) as tc:
        with tc.tile_pool(name="persist", bufs=1) as pp:
            # ---- persistent SBUF ----
            wqk_sb = pp.tile([128, 6, 2 * DH * NH], bf16)
            bqk_sb = pp.tile([128, NH], f32)
            wv_sb = pp.tile([128, 6, DH * NH], bf16)
            bv_sb = pp.tile([1, DH * NH], bf16)
            woutp_sb = pp.tile([128, 2, D], bf16)
            rel4_sb = pp.tile([128, NH, 4, 512], f32)
            ones_sb = pp.tile([65, 128], bf16)  # rows 0/32/64 used
            V_sb = pp.tile([128, KT, NH, DH + 1], bf16)
            ctxA = pp.tile([128, L], bf16)  # heads 0,1 of group (unnorm ctx^T)
            ctxB = pp.tile([64, L], bf16)  # head 2 of group
            # softmax denominators / reciprocals: head j lives on partition
            # 32*j so the K=1 broadcast matmul sees a legal base partition.
            den_sb = pp.tile([65, L], f32)
            rec_sb = pp.tile([65, L], bf16)
            # Per-head attention operand buffers (fp32r).
            # Kbuf rows: 0-63 K^T, 64 mask, 65 s*(k-1024), 66 s
            # QbufR rows: 0-63 Q'^T, 64 1, 65 1, 66 -(q-1024)
            # QbufL rows: 0-63 Q'^T, 64 1, 65 -1, 66 +(q-1024)
            Kbuf = [
                pp.tile([67, L], f32r, tag=f"kb{j}", name=f"kb{j}") for j in range(NH)
            ]
            QbufR = [
                pp.tile([67, L], f32r, tag=f"qr{j}", name=f"qr{j}") for j in range(NH)
            ]
            QbufL = [
                pp.tile([67, L], f32r, tag=f"ql{j}", name=f"ql{j}") for j in range(NH)
            ]

            # Stage-1-critical DMAs first (wqk/x feed the first matmuls);
            # later-needed tensors go on the gpsimd DMA queue so they don't
            # delay the compute-critical loads.
            nc.sync.dma_start(wqk_sb[:], wqk_d.rearrange("(o p) m -> p o m", p=128))
            nc.sync.dma_start(bqk_sb[:], bqk_d[:])
            nc.sync.dma_start(wv_sb[:], wv_d.rearrange("(o p) m -> p o m", p=128))
            nc.sync.dma_start(bv_sb[:], bv_d[:])
            nc.vector.memset(ones_sb[:], 1.0)
            nc.gpsimd.memset(V_sb[:, :, :, DH : DH + 1], 1.0)
            # unused den rows must stay finite through the batched Ln/Exp
            nc.gpsimd.memset(den_sb[:], 1.0)
            for j in range(NH):
                nc.gpsimd.dma_start(QbufR[j][64:67, :], augqR_d[:])
                nc.gpsimd.dma_start(QbufL[j][64:67, :], augqL_d[:])
                nc.gpsimd.dma_start(Kbuf[j][64:67, :], augk_d[j])
            nc.gpsimd.dma_start(rel4_sb[:], rel4_d[:])
            nc.gpsimd.dma_start(
                woutp_sb[:], woutp_d.rearrange("(o p) n -> p o n", p=128)
            )

            # Pre-touch DMA/memset-filled tiles with the engine that later
            # writes other rows of the same tile, so those writes carry a
            # single sync wait (walrus TS encoding rejects multi-wait).
            junk = pp.tile([1, 16], f32, name="junk")
            nc.vector.tensor_copy(junk[0:1, 0:1], bqk_sb[0:1, 0:1])
            nc.vector.tensor_copy(junk[0:1, 1:2], rel4_sb[0:1, 0, 0, 0:1])
            junkr = pp.tile([1, 16], f32r, name="junkr")
            junkg = pp.tile([1, 16], f32r, name="junkg")
            for j in range(NH):
                nc.vector.tensor_copy(junkr[0:1, 2 + j : 3 + j], QbufR[j][64:65, 0:1])
                nc.gpsimd.tensor_copy(junkg[0:1, 5 + j : 6 + j], QbufL[j][64:65, 0:1])
                nc.vector.tensor_copy(junkr[0:1, 8 + j : 9 + j], Kbuf[j][64:65, 0:1])
            junkb = pp.tile([1, 4], bf16, name="junkb")
            nc.vector.tensor_copy(junkb[0:1, 0:1], V_sb[0:1, 0, 0, DH : DH + 1])

            # ---- stage 1: QKV projection (bf16 matmuls) ----
            with (
                tc.tile_pool(name="xpool", bufs=1) as xp,
                tc.tile_pool(name="ps1", bufs=4, space="PSUM") as ps1,
            ):
                xT_sb = xp.tile([128, 6, L], bf16)
                for kt in range(6):
                    nc.sync.dma_start(
                        xT_sb[:, kt, :],
                        xT_d.rearrange("(o p) f -> p o f", p=128)[:, kt, :],
                    )

                # Q^T/K^T per head: PSUM [128, 512] = [Q^T_h/8; K^T_h] chunk
                # (the 1/8 scale is folded into wqk/bqk host-side)
                for j in range(NH):
                    pcs = [
                        ps1.tile([128, 512], f32, tag="ps1", name=f"ps1c{c}")
                        for c in range(QC)
                    ]
                    for kt in range(6):
                        for c in range(QC):
                            nc.tensor.matmul(
                                pcs[c],
                                wqk_sb[:, kt, ts(j, 128)],
                                xT_sb[:, kt, ts(c, 512)],
                                start=(kt == 0),
                                stop=(kt == 5),
                            )
                    for c in range(QC):
                        ps = pcs[c]
                        cs = ts(c, 512)
                        nc.vector.tensor_scalar(
                            QbufR[j][0:64, cs],
                            ps[0:64, :],
                            bqk_sb[0:64, j : j + 1],
                            None,
                            ADD,
                        )
                        nc.gpsimd.tensor_copy(QbufL[j][0:64, cs], QbufR[j][0:64, cs])
                        nc.vector.tensor_scalar(
                            Kbuf[j][0:64, cs],
                            ps[64:128, :],
                            bqk_sb[64:128, j : j + 1],
                            None,
                            ADD,
                        )

                # V natural layout [l, d] + bias via K=1 matmul
                for lt in range(KT):
                    psv = ps1.tile([128, 512], f32, tag="ps1", name="psv")[:, : DH * NH]
                    for kt in range(6):
                        nc.tensor.matmul(
                            psv,
                            xT_sb[:, kt, ts(lt, 128)],
                            wv_sb[:, kt, :],
                            start=(kt == 0),
                            stop=False,
                        )
                    nc.tensor.matmul(
                        psv,
                        ones_sb[0:1, 0:128],
                        bv_sb[0:1, :],
                        start=False,
                        stop=True,
                    )
                    nc.vector.tensor_copy(
                        V_sb[:, lt, :, 0:DH],
                        psv.rearrange("p (h x) -> p h x", x=DH),
                    )

            # ---- stage 2: attention ----
            with (
                tc.tile_pool(name="psS", bufs=2, space="PSUM") as psS,
                tc.tile_pool(name="psO", bufs=2, space="PSUM") as psO,
                tc.tile_pool(name="ptp", bufs=3) as ptp,
            ):
                for j in range(NH):
                    for c in range(QC):
                        cs = ts(c, 512)
                        t_lo, t_hi = BANDS[j][c]
                        out_t = psO.tile([128, 512], f32, tag="outaug")
                        for t0 in range(t_lo, t_hi, GROUP_SIZE):
                            tn = min(GROUP_SIZE, t_hi - t0)
                            st = psS.tile([128, GROUP_SIZE * 512], f32, tag="st")
                            for i in range(tn):
                                t = t0 + i
                                js = ts(i, 512)
                                m = t - 4 * c
                                if 0 <= m < 4:  # diagonal block
                                    nc.tensor.matmul(
                                        st[:, js],
                                        Kbuf[j][0:65, ts(t, 128)],
                                        QbufR[j][0:65, cs],
                                        start=True,
                                        stop=True,
                                    )
                                    nc.vector.scalar_tensor_tensor(
                                        st[:, js],
                                        rel4_sb[:, j, m, :],
                                        1.0,
                                        st[:, js],
                                        MULT,
                                        ADD,
                                    )
                                elif c > t // 4:  # k < q: aug gives -s*(q-k)
                                    nc.tensor.matmul(
                                        st[:, js],
                                        Kbuf[j][0:67, ts(t, 128)],
                                        QbufR[j][0:67, cs],
                                        start=True,
                                        stop=True,
                                    )
                                else:  # k > q: aug gives -s*(k-q)
                                    nc.tensor.matmul(
                                        st[:, js],
                                        Kbuf[j][0:67, ts(t, 128)],
                                        QbufL[j][0:67, cs],
                                        start=True,
                                        stop=True,
                                    )
                            pt = ptp.tile([128, GROUP_SIZE * 512], bf16, tag="pt")
                            nc.scalar.activation(
                                pt[:, : tn * 512], st[:, : tn * 512], Exp
                            )
                            for i in range(tn):
                                t = t0 + i
                                nc.tensor.matmul(
                                    out_t[0 : DH + 1, :],
                                    V_sb[:, t, j, :],
                                    pt[:, ts(i, 512)],
                                    start=(t == t_lo),
                                    stop=(t == t_hi - 1),
                                    skip_group_check=True,
                                )
                        # stage unnormalized ctx + denominator to SBUF
                        if j < 2:
                            ctx_slice = ctxA[j * 64 : (j + 1) * 64, cs]
                        else:
                            ctx_slice = ctxB[0:64, cs]
                        nc.vector.tensor_copy(ctx_slice, out_t[0:DH, :])
                        nc.vector.tensor_copy(
                            den_sb[32 * j : 32 * j + 1, cs], out_t[DH : DH + 1, :]
                        )

            # ---- stage 3: normalization + output projection ----
            with (
                tc.tile_pool(name="ps3", bufs=2, space="PSUM") as ps3,
                tc.tile_pool(name="psR", bufs=2, space="PSUM") as psR,
                tc.tile_pool(name="ysb", bufs=3) as yp,
            ):
                # 1/denom = exp(-ln(denom)), one batched pass (in-place Ln;
                # Ln and Exp share the natural_log_exp ACT table set).
                nc.scalar.activation(den_sb[:], den_sb[:], Ln)
                nc.scalar.activation(rec_sb[:], den_sb[:], Exp, scale=-1.0)

                # broadcast 1/denom across 64 partitions via K=1 matmuls,
                # then normalize ctx in place against the PSUM broadcast.
                for j in range(NH):
                    for c in range(QC):
                        cs = ts(c, 512)
                        recb_ps = psR.tile([64, 512], f32, tag="recb")
                        nc.tensor.matmul(
                            recb_ps,
                            ones_sb[32 * j : 32 * j + 1, 0:64],
                            rec_sb[32 * j : 32 * j + 1, cs],
                            start=True,
                            stop=True,
                        )
                        if j < 2:
                            ctx_slice = ctxA[j * 64 : (j + 1) * 64, cs]
                        else:
                            ctx_slice = ctxB[0:64, cs]
                        nc.vector.tensor_mul(ctx_slice, ctx_slice, recb_ps)

                for lt in range(KT):
                    y = yp.tile([128, D], f32, tag="y")
                    for n0, nw in ((0, 512), (512, 256)):
                        ps = ps3.tile([128, 512], f32, tag="ps3", name="ps3t")[:, :nw]
                        nc.tensor.matmul(
                            ps,
                            ctxA[:, ts(lt, 128)],
                            woutp_sb[:, 0, n0 : n0 + nw],
                            start=True,
                            stop=False,
                        )
                        nc.tensor.matmul(
                            ps,
                            ctxB[0:64, ts(lt, 128)],
                            woutp_sb[0:64, 1, n0 : n0 + nw],
                            start=False,
                            stop=True,
                        )
                        if n0 == 0:
                            nc.scalar.copy(y[:, n0 : n0 + nw], ps)
                        else:
                            nc.vector.tensor_copy(y[:, n0 : n0 + nw], ps)
                    nc.sync.dma_start(y_d[ts(lt, 128), :], y)

    if not nc.is_finalized():
        nc.finalize()
    _PROGRAM_CACHE["nc"] = nc
    return nc


def _host_inputs(x, attn_mask, Wqkv, bqkv, Wout, bout):
    """Build the 8 per-core input dicts."""
    import ml_dtypes

    bf16 = ml_dtypes.bfloat16
    slopes = alibi_slopes(H)
    x = np.asarray(x, dtype=np.float32)
    attn_mask = np.asarray(attn_mask)
    Wqkv = np.asarray(Wqkv, dtype=np.float32)
    bqkv = np.asarray(bqkv, dtype=np.float32)
    Wout = np.asarray(Wout, dtype=np.float32)
    bout = np.asarray(bout, dtype=np.float32)

    q_idx = np.arange(L, dtype=np.float32)
    ones_row = np.ones(L, dtype=np.float32)
    qrel = q_idx - QCENTER
    augqR = np.ascontiguousarray(np.stack([ones_row, ones_row, -qrel]))
    augqL = np.ascontiguousarray(np.stack([ones_row, -ones_row, qrel]))

    # rel4[p, m, q'] = |q' - 128m - p|  (diagonal-block relative distance)
    p = np.arange(128, dtype=np.float32)[:, None, None]
    m = np.arange(4, dtype=np.float32)[None, :, None]
    qq = np.arange(512, dtype=np.float32)[None, None, :]
    rel4_base = np.abs(qq - 128.0 * m - p).astype(np.float32)

    in_maps = []
    for core in range(N_CORES):
        b = core // 4
        g = core % 4
        heads = HEAD_GROUPS[g]

        wqk = np.empty((D, 2 * DH * NH), np.float32)
        bqk = np.empty((128, NH), np.float32)
        wv = np.empty((D, DH * NH), np.float32)
        bv = np.empty((1, DH * NH), np.float32)
        woutp = np.zeros((256, D), np.float32)
        augk = np.empty((NH, 3, L), np.float32)
        mask_row = np.where(attn_mask[b] == 0, NEG_MASK, 0.0).astype(np.float32)
        rel4 = np.empty((128, NH, 4, 512), np.float32)
        for jj, h in enumerate(heads):
            rel4[:, jj] = -float(slopes[h]) * rel4_base
            # Q half pre-scaled by 1/8 = 1/sqrt(DH)
            wqk[:, jj * 128 : jj * 128 + 64] = Wqkv[:, h * DH : (h + 1) * DH] * 0.125
            wqk[:, jj * 128 + 64 : (jj + 1) * 128] = Wqkv[
                :, D + h * DH : D + (h + 1) * DH
            ]
            bqk[0:64, jj] = bqkv[h * DH : (h + 1) * DH] * 0.125
            bqk[64:128, jj] = bqkv[D + h * DH : D + (h + 1) * DH]
            wv[:, jj * DH : (jj + 1) * DH] = Wqkv[
                :, 2 * D + h * DH : 2 * D + (h + 1) * DH
            ]
            bv[0, jj * DH : (jj + 1) * DH] = bqkv[2 * D + h * DH : 2 * D + (h + 1) * DH]
            woutp[jj * DH : (jj + 1) * DH, :] = Wout[h * DH : (h + 1) * DH, :]
            s = float(slopes[h])
            augk[jj, 0, :] = mask_row
            augk[jj, 1, :] = s * qrel  # s * (k_idx - 1024)
            augk[jj, 2, :] = s
        in_maps.append(
            {
                "xT": np.ascontiguousarray(x[b].T).astype(bf16),
                "wqk": wqk.astype(bf16),
                "bqk": bqk,
                "wv": wv.astype(bf16),
                "bv": bv.astype(bf16),
                "woutp": woutp.astype(bf16),
                "augqR": augqR,
                "augqL": augqL,
                "augk": augk,
                "rel4": rel4,
            }
        )
    return in_maps


def kernel(x, attn_mask, Wqkv, bqkv, Wout, bout):
    _ensure_concourse()
    from concourse.bass_utils import run_bass_kernel_spmd

    nc = _build_program()
    in_maps = _host_inputs(x, attn_mask, Wqkv, bqkv, Wout, bout)

    res = run_bass_kernel_spmd(
        nc,
        in_maps,
        list(range(N_CORES)),
        trace=bool(os.environ.get("BASS_TRACE")),
    )
    outs = [r["ypart"] for r in res.results]
    out = np.zeros((B, L, D), np.float32)
    for core in range(N_CORES):
        out[core // 4] += outs[core]
    out += np.asarray(bout, np.float32)[None, None, :]
    if res.exec_time_ns is not None:
        kernel.last_exec_time_ns = res.exec_time_ns
    kernel.last_result = res
    return out
